# revision 43
# baseline (speedup 1.0000x reference)
"""Local multi-head attention (window=128, look 1/1) on 8 trn2 cores.

Sharding: core c owns (batch b = c//2, sequence half h = c%2) = 1024 query
positions; k/v are recomputed over a 1280-position halo slice (one extra
128-window each side, zero-padded at the global sequence edges).

Attention runs in transposed-score orientation to avoid PE transposes:
  scoresT[j, i] = k_j . q_i  per (head, key-window u), with the query block
  being the up-to-3 query windows that attend key-window u. exp() runs on
  the scalar engine (PSUM -> SBUF bf16); softmax denominators come from
  col-tiled ones-matmuls into the same partition rows as the output
  accumulator, so the normalize is two aligned vector ops. Zero-padded
  edge key-windows would contribute exp(0)=1 per fake key, so their
  exp-scores are multiplied by a per-core 0/1 mask (reproducing the
  reference's -inf edge masking).

Head pairs (2t, 2t+1) share feature tile t: scores matmuls are row-tiled
(K=64 at array rows 0:64 / 64:128), attention@V and denominator matmuls
are col-tiled (M=64 writing partitions 0:64 / 64:128), so both heads run
concurrently in the PE array.

Schedule (v4): warmup matmuls ride out the HAM cold window during the
initial (interleaved w/x) DMAs; phases are k-proj, q-proj, then a fused
loop where v-proj chunk m, attention block m-2, scores block m, and the
first-half (k=0..3) out-projection partials are interleaved so the
scalar-engine exp stream always has dependency-free PE work beside it.
The out-projection tail is only the k=4..7 half plus an add of the
bf16 partial. Softmax normalize uses reciprocal_approx_fast.
"""

import sys

sys.path.insert(0, "/opt/trn_rl_repo")

import ml_dtypes
import numpy as np

import concourse.bass as bass
import concourse.mybir as mybir
from concourse import bacc
from concourse.bass_utils import run_bass_kernel_spmd
from concourse.tile import TileContext

S, B, E, H, HD, W = 2048, 4, 1024, 16, 64, 128
NC = 8
SC = S // 2           # 1024 positions per core (one batch, half the sequence)
SH = SC + 2 * W       # 1280 halo positions = 10 key windows
NW = SC // W          # 8 owned query windows
NKW = SH // W         # 10 key windows (halo coords); owned = 1..8
KT = E // 128         # 8 contraction tiles
F32 = mybir.dt.float32
BF16 = mybir.dt.bfloat16

_COMPILED = {}


def _qblock(u):
    """Query-window range (halo coords, clipped to owned 1..NW) attending
    key-window u, as column range [qc0, qc1) of the 1024 owned queries."""
    wlo, whi = max(u - 1, 1), min(u + 1, NW)
    return (wlo - 1) * W, whi * W


def _build_nc():
    nc = bacc.Bacc("TRN2", target_bir_lowering=False, debug=False, num_devices=NC)
    xq = nc.dram_tensor("xq", [E, SC], BF16, kind="ExternalInput").ap()
    xk = nc.dram_tensor("xk", [E, SH], BF16, kind="ExternalInput").ap()
    xv = nc.dram_tensor("xv", [E, SH], BF16, kind="ExternalInput").ap()
    wq = nc.dram_tensor("wq", [E, E], BF16, kind="ExternalInput").ap()
    wk = nc.dram_tensor("wk", [E, E], BF16, kind="ExternalInput").ap()
    wv = nc.dram_tensor("wv", [E, E], BF16, kind="ExternalInput").ap()
    wo = nc.dram_tensor("wo", [E, E], BF16, kind="ExternalInput").ap()
    bo = nc.dram_tensor("bo", [KT, 128, 1], F32, kind="ExternalInput").ap()
    maskL = nc.dram_tensor("maskL", [128, 1], F32, kind="ExternalInput").ap()
    maskR = nc.dram_tensor("maskR", [128, 1], F32, kind="ExternalInput").ap()
    out = nc.dram_tensor("out", [E, SC], F32, kind="ExternalOutput").ap()

    with TileContext(nc) as tc:
        with (
            tc.tile_pool(name="persist", bufs=1) as pp,
            tc.tile_pool(name="wx", bufs=2) as wx,
            tc.tile_pool(name="ppj", bufs=2, space="PSUM") as ppj,
            tc.tile_pool(name="scp", bufs=2, space="PSUM") as scp,
            tc.tile_pool(name="pop", bufs=2, space="PSUM") as pop,
            tc.tile_pool(name="dnp", bufs=2, space="PSUM") as dnp,
            tc.tile_pool(name="esc", bufs=47) as esc,
            tc.tile_pool(name="rcp", bufs=2) as rcp,
            tc.tile_pool(name="ot", bufs=3) as otp,
        ):
            def load_wx(w_dram, x_dram, n_cols):
                """Interleave weight-tile and first-chunk x DMAs so the k=0..7
                accumulation chain can begin after ~one tile-pair arrives."""
                w_sb = [wx.tile([128, E], BF16, name=f"w{k}", tag=f"w{k}") for k in range(KT)]
                x_sb = [wx.tile([128, SH], BF16, name=f"x{k}", tag=f"x{k}") for k in range(KT)]
                for k in range(KT):
                    nc.sync.dma_start(out=w_sb[k][:], in_=w_dram[bass.ts(k, 128), :])
                    nc.sync.dma_start(out=x_sb[k][:, 0:512], in_=x_dram[bass.ts(k, 128), 0:512])
                for k in range(KT):
                    nc.sync.dma_start(
                        out=x_sb[k][:, 512:n_cols], in_=x_dram[bass.ts(k, 128), 512:n_cols]
                    )
                return w_sb, x_sb

            def load_w(w_dram):
                w_sb = [wx.tile([128, E], BF16, name=f"w{k}", tag=f"w{k}") for k in range(KT)]
                for k in range(KT):
                    nc.sync.dma_start(out=w_sb[k][:], in_=w_dram[bass.ts(k, 128), :])
                return w_sb

            def load_x(x_dram, n_cols):
                x_sb = [wx.tile([128, SH], BF16, name=f"x{k}", tag=f"x{k}") for k in range(KT)]
                for k in range(KT):
                    nc.sync.dma_start(out=x_sb[k][:, 0:512], in_=x_dram[bass.ts(k, 128), 0:512])
                for k in range(KT):
                    nc.sync.dma_start(
                        out=x_sb[k][:, 512:n_cols], in_=x_dram[bass.ts(k, 128), 512:n_cols]
                    )
                return x_sb

            # persistent tiles
            qT = [pp.tile([128, SC], BF16, name=f"qT{m}", tag=f"qT{m}") for m in range(KT)]
            kT = [pp.tile([128, SH], BF16, name=f"kT{m}", tag=f"kT{m}") for m in range(KT)]
            vb = [pp.tile([128, E], BF16, name=f"v{m}", tag=f"v{m}") for m in range(NKW)]
            aoT = [pp.tile([128, SC], BF16, name=f"aoT{m}", tag=f"aoT{m}") for m in range(KT)]
            pb = [pp.tile([128, SC], BF16, name=f"pb{m}", tag=f"pb{m}") for m in range(KT)]
            bo_sb = [pp.tile([128, 1], F32, name=f"bo{g}", tag=f"bo{g}") for g in range(KT)]
            mL = pp.tile([128, 1], F32, name="mL", tag="mL")
            mR = pp.tile([128, 1], F32, name="mR", tag="mR")
            ones = pp.tile([128, 64], BF16, name="ones", tag="ones")
            warm = pp.tile([128, 512], BF16, name="warm", tag="warm")

            # ---- HAM warmup: dependency-free matmuls on memset data ----
            nc.vector.memset(ones[:], 1.0)
            nc.vector.memset(warm[:], 0.0)
            for _ in range(3):
                ps = ppj.tile([128, 512], F32, name="ps", tag="ps")
                for r in range(4):
                    nc.tensor.matmul(ps[:], warm[:, 0:128], warm[:], start=(r == 0), stop=(r == 3))

            def filler(nf):
                """Dependency-free matmuls into an unused score bank: keep the
                HAM clock warm while DMA-gated chains trickle in."""
                wp = scp.tile([128, 384], F32, name="sc", tag="sc")
                for r in range(nf):
                    nc.tensor.matmul(
                        wp[:], warm[:, 0:128], warm[:, 0:384],
                        start=(r == 0), stop=(r == nf - 1),
                    )

            # ---- k projection (feature-major kT[t][128, 1280]) ----
            wk_sb, xk_sb = load_wx(wk, xk, SH)
            for g in range(KT):
                nc.sync.dma_start(out=bo_sb[g][:], in_=bo[g])
            nc.sync.dma_start(out=mL[:], in_=maskL)
            nc.sync.dma_start(out=mR[:], in_=maskR)

            for m in range(KT):
                for i, (c0, c1) in enumerate(((0, 512), (512, 1024), (1024, 1280))):
                    ps = ppj.tile([128, 512], F32, name="ps", tag="ps")
                    for k in range(KT):
                        nc.tensor.matmul(
                            ps[:, 0 : c1 - c0],
                            wk_sb[k][:, bass.ts(m, 128)],
                            xk_sb[k][:, c0:c1],
                            start=(k == 0),
                            stop=(k == KT - 1),
                        )
                        if m == 0 and i == 0:
                            filler(2)
                        elif m == 0 and i == 1:
                            filler(1)
                    if (m + i) % 2:
                        nc.vector.tensor_copy(kT[m][:, c0:c1], ps[:, 0 : c1 - c0])
                    else:
                        nc.scalar.copy(kT[m][:, c0:c1], ps[:, 0 : c1 - c0])

            # ---- q projection (feature-major qT[t][128, 1024]) ----
            wq_sb = [wx.tile([128, E], BF16, name=f"w{k}", tag=f"w{k}") for k in range(KT)]
            xq_sb = [wx.tile([128, SH], BF16, name=f"x{k}", tag=f"x{k}") for k in range(KT)]
            for k in range(KT):
                nc.sync.dma_start(out=wq_sb[k][:], in_=wq[bass.ts(k, 128), :])
                nc.sync.dma_start(out=xq_sb[k][:, 0:512], in_=xq[bass.ts(k, 128), 0:512])
            for k in range(KT):
                nc.sync.dma_start(out=xq_sb[k][:, 512:SC], in_=xq[bass.ts(k, 128), 512:SC])
            for m in range(KT):
                for n in range(2):
                    ps = ppj.tile([128, 512], F32, name="ps", tag="ps")
                    for k in range(KT):
                        nc.tensor.matmul(
                            ps[:],
                            wq_sb[k][:, bass.ts(m, 128)],
                            xq_sb[k][:, bass.ts(n, 512)],
                            start=(k == 0),
                            stop=(k == KT - 1),
                        )
                    if (m + n) % 2:
                        nc.vector.tensor_copy(qT[m][:, bass.ts(n, 512)], ps[:])
                    else:
                        nc.scalar.copy(qT[m][:, bass.ts(n, 512)], ps[:])

            # ---- v inputs + wo prefetch ----
            wv_sb, xv_sb = load_wx(wv, xv, SH)
            wo_sb = load_w(wo)

            et = [{} for _ in range(KT)]

            def scores_block(t):
                for u in range(NKW):
                    qc0, qc1 = _qblock(u)
                    n = qc1 - qc0
                    eu = []
                    for hh, r0 in ((0, 0), (1, 64)):
                        sc = scp.tile([128, 384], F32, name="sc", tag="sc")
                        nc.tensor.matmul(
                            sc[:, 0:n],
                            kT[t][r0 : r0 + 64, bass.ts(u, 128)],
                            qT[t][r0 : r0 + 64, qc0:qc1],
                            start=True,
                            stop=True,
                        )
                        ee = esc.tile([128, 384], BF16, name="ee", tag="ee")
                        nc.scalar.activation(
                            ee[:, 0:n],
                            sc[:, 0:n],
                            mybir.ActivationFunctionType.Exp,
                            scale=float(HD) ** -0.5,
                        )
                        if u == 0:
                            nc.vector.tensor_scalar_mul(ee[:, 0:n], ee[:, 0:n], mL[:])
                        elif u == NKW - 1:
                            nc.vector.tensor_scalar_mul(ee[:, 0:n], ee[:, 0:n], mR[:])
                        eu.append(ee)
                    et[t][u] = (eu[0], 0, eu[1], 0, qc0, qc1)

            def attn_block(t):
                """A@V + denominator + normalize for feature tile t (heads
                2t, 2t+1), both query halves."""
                for qh in range(2):
                    g0, g1 = qh * 512, qh * 512 + 512
                    us = [u for u in range(NKW) if et[t][u][4] < g1 and et[t][u][5] > g0]
                    po = pop.tile([128, 512], F32, name="po", tag="po")
                    den = dnp.tile([128, 512], F32, name="den", tag="den")
                    for i, u in enumerate(us):
                        eA, offA, eB, offB, qc0, qc1 = et[t][u]
                        c0, c1 = max(qc0, g0), min(qc1, g1)
                        start, stop = (i == 0), (i == len(us) - 1)
                        for hh, (r0, ee, off) in enumerate(((0, eA, offA), (64, eB, offB))):
                            rhs = ee[:, off + c0 - qc0 : off + c1 - qc0]
                            nc.tensor.matmul(
                                po[r0 : r0 + 64, c0 - g0 : c1 - g0],
                                vb[u][:, (2 * t + hh) * 64 : (2 * t + hh + 1) * 64],
                                rhs,
                                start=start,
                                stop=stop,
                            )
                            nc.tensor.matmul(
                                den[r0 : r0 + 64, c0 - g0 : c1 - g0],
                                ones[:, 0:64],
                                rhs,
                                start=start,
                                stop=stop,
                            )
                    rec = rcp.tile([128, 512], F32, name="rec", tag="rec")
                    nc.vector.reciprocal_approx_fast(out=rec[:], in_=den[:])
                    nc.vector.tensor_mul(aoT[t][:, g0:g1], po[:], rec[:])

            def outproj_partial(mo, n):
                """First-half contraction (k=0..3) of out tile mo, query half
                n, accumulated to a bf16 partial in SBUF."""
                ps = ppj.tile([128, 512], F32, name="ps", tag="ps")
                for k in range(4):
                    nc.tensor.matmul(
                        ps[:],
                        wo_sb[k][:, bass.ts(mo, 128)],
                        aoT[k][:, bass.ts(n, 512)],
                        start=(k == 0),
                        stop=(k == 3),
                    )
                nc.vector.tensor_copy(pb[mo][:, bass.ts(n, 512)], ps[:])

            # ---- v projection fused with the scores stream ----
            for m in range(NKW):
                if m < KT:
                    scores_block(m)
                for n in range(2):
                    ps = ppj.tile([128, 512], F32, name="ps", tag="ps")
                    for k in range(KT):
                        nc.tensor.matmul(
                            ps[:],
                            xv_sb[k][:, bass.ts(m, 128)],
                            wv_sb[k][:, bass.ts(n, 512)],
                            start=(k == 0),
                            stop=(k == KT - 1),
                        )
                    nc.vector.tensor_copy(vb[m][:, bass.ts(n, 512)], ps[:])

            for t in range(4):
                attn_block(t)
            for t in range(4, KT):
                attn_block(t)
                for mo in ((t - 4) * 2, (t - 4) * 2 + 1):
                    outproj_partial(mo, 0)
                    outproj_partial(mo, 1)

            # ---- output projection + bias ----
            for m in range(KT):
                for n in range(2):
                    ps = ppj.tile([128, 512], F32, name="ps", tag="ps")
                    for k in range(4, KT):
                        nc.tensor.matmul(
                            ps[:],
                            wo_sb[k][:, bass.ts(m, 128)],
                            aoT[k][:, bass.ts(n, 512)],
                            start=(k == 4),
                            stop=(k == KT - 1),
                        )
                    nc.vector.tensor_add(ps[:], ps[:], pb[m][:, bass.ts(n, 512)])
                    ot = otp.tile([128, 512], F32, name="ot", tag="ot")
                    nc.scalar.activation(
                        ot[:],
                        ps[:],
                        mybir.ActivationFunctionType.Identity,
                        bias=bo_sb[m][:],
                    )
                    nc.sync.dma_start(
                        out=out[bass.ts(m, 128), bass.ts(n, 512)], in_=ot[:]
                    )

    nc.finalize()
    return nc


def _shard_inputs(query, key, value, Wq, Wk, Wv, Wo, bo, key_padding_mask):
    del key_padding_mask  # all-False in this problem; exact by construction
    q = np.asarray(query, np.float32)
    k = np.asarray(key, np.float32)
    v = np.asarray(value, np.float32)
    wqT = np.ascontiguousarray(np.asarray(Wq, np.float32).T.astype(ml_dtypes.bfloat16))
    wkT = np.ascontiguousarray(np.asarray(Wk, np.float32).T.astype(ml_dtypes.bfloat16))
    wvT = np.ascontiguousarray(np.asarray(Wv, np.float32).T.astype(ml_dtypes.bfloat16))
    woT = np.ascontiguousarray(np.asarray(Wo, np.float32).T.astype(ml_dtypes.bfloat16))
    bo_r = np.ascontiguousarray(np.asarray(bo, np.float32).reshape(KT, 128, 1))
    in_maps = []
    for c in range(NC):
        b, h = c // 2, c % 2
        s0 = h * SC
        xqT = np.ascontiguousarray(q[s0 : s0 + SC, b, :].T.astype(ml_dtypes.bfloat16))
        kh = np.zeros((SH, E), np.float32)
        vh = np.zeros((SH, E), np.float32)
        lo, hi = s0 - W, s0 + SC + W
        glo, ghi = max(lo, 0), min(hi, S)
        kh[glo - lo : ghi - lo] = k[glo:ghi, b, :]
        vh[glo - lo : ghi - lo] = v[glo:ghi, b, :]
        xkT = np.ascontiguousarray(kh.T.astype(ml_dtypes.bfloat16))
        xvT = np.ascontiguousarray(vh.T.astype(ml_dtypes.bfloat16))
        mLc = np.full((128, 1), 0.0 if h == 0 else 1.0, np.float32)
        mRc = np.full((128, 1), 0.0 if h == 1 else 1.0, np.float32)
        in_maps.append(
            {
                "xq": xqT, "xk": xkT, "xv": xvT,
                "wq": wqT, "wk": wkT, "wv": wvT, "wo": woT,
                "bo": bo_r, "maskL": mLc, "maskR": mRc,
            }
        )
    return in_maps


def kernel(**inputs) -> np.ndarray:
    if "nc" not in _COMPILED:
        _COMPILED["nc"] = _build_nc()
    nc = _COMPILED["nc"]
    in_maps = _shard_inputs(**inputs)
    res = run_bass_kernel_spmd(nc, in_maps, list(range(NC)))
    out = np.empty((S, B, E), np.float32)
    for c in range(NC):
        b, h = c // 2, c % 2
        s0 = h * SC
        out[s0 : s0 + SC, b, :] = res.results[c]["out"].T
    return out
